# revision 58
# baseline (speedup 1.0000x reference)
"""Trainium2 Bass kernel for nn_CrossAttention (B=4, C=512, H=W=64, CQK=64).

Math (per batch b):
    Q = Wq @ rgb + bq                      [CQK, HW]
    K = Wk @ chm + bk                      [CQK, XY]
    S[hw, xy] = sum_o Q[o, hw] K[o, xy]
    P = softmax over y only (xy = x*64 + y)
    att[c, hw] = sum_xy P[hw, xy] V[c, xy],  V = Wv @ chm + bv
    out = rgb + gamma * att

Sharding: 8 cores = 4 batches x 2 halves of the hw (query) axis; weights
replicated; no collectives.

Device dataflow per core (vs the bf16 baseline):
  - S columns are stored y-major ("flip"): column j = y*64 + x, arranged by
    permuting the PSUM->SBUF write APs of K (and chm's fp8 copy). Softmax
    groups (fixed x, all y) are then stride-64 column sets; the Z tree and
    the bf16 normalize multiplies keep stride-1 innermost APs (DVE 2x).
  - M1 = chm @ P^T runs in fp8e4 DoubleRow mode (2 contraction rows per PE
    cell). Both operands are transposed as *uint16 pair views* of fp8
    tiles through the DMA xbar (fp8 alone is not transposable); the pair
    (x=2xp, x=2xp+1 | y) indexing is identical on both sides and the
    matmul's [K, 2, N] APs un-interleave pairs with stride-2 fp8 dims.
  - P normalization: 1 of 4 htiles quantizes to fp8 directly on DVE; the
    other 3 normalize bf16 (2x mode) into a staging tile and gpsimd does
    the fp8 convert, balancing DVE against the Pool engine. PSUM drains
    ride DVE so ACT carries only the exp stream.
  - Attend lags softmax by 1.5 blocks (depth-1.5): softmax(b) hosts the
    second half of attend(b-2), its fini, then the first half of
    attend(b-1), so DoubleRow matmuls interleaved into softmax's
    exp-paced S gaps never wait on P^T and the tail stays short. Output
    stores go out as halves right after each drain.
  - rgb stays resident (bf16) from phase 1; the rgb add rides an
    identity matmul accumulated into the att2 PSUM group.
  - All loads are priority-ordered on the ACT HWDGE ring (chm half
    first; the per-core HBM port serializes them anyway); transposes
    ride the SP ring; kf drains per 1024-col group as K accumulation
    completes. att2 stays bf16; gamma*Wv and 64*gamma*bv fold on host.
"""

import numpy as np

import concourse.bass as bass
import concourse.mybir as mybir
import concourse.tile as tile
from concourse import bacc
from concourse.bass_utils import run_bass_kernel_spmd

P = 128
B, C, H, W = 4, 512, 64, 64
HW = H * W                # 4096
CQK = C // 8              # 64
N_CORES = 8
HWC = HW // 2             # hw rows per core (2048)

F32 = mybir.dt.float32
F32R = mybir.dt.float32r
BF16 = mybir.dt.bfloat16
FP8 = mybir.dt.float8e4
U16 = mybir.dt.uint16
ADD = mybir.AluOpType.add
MULT = mybir.AluOpType.mult
IDENT = mybir.ActivationFunctionType.Identity
EXP = mybir.ActivationFunctionType.Exp
DR = mybir.MatmulPerfMode.DoubleRow


def build_program(hwc=HWC, xy=HW, c=C, cqk=CQK, n_cores=N_CORES):
    """Build the per-core Bass program. Returns a compiled Bacc module."""
    ck = c // P               # channel chunks (4)
    nb = hwc // 512           # hw blocks (4)
    y = 64                    # softmax group size
    nx = xy // y              # x values (64)
    jt = xy // 2 // P         # u16-pair transpose tiles over xy (16)

    nc = bacc.Bacc("TRN2", target_bir_lowering=False, debug=False,
                   num_devices=n_cores)

    rgb = nc.dram_tensor("rgb", [c, hwc], F32, kind="ExternalInput")
    chm = nc.dram_tensor("chm", [c, xy], F32, kind="ExternalInput")
    wqT = nc.dram_tensor("wqT", [c, 2 * cqk], F32, kind="ExternalInput")
    wkT = nc.dram_tensor("wkT", [c, 2 * cqk], F32, kind="ExternalInput")
    wvT = nc.dram_tensor("wvT", [c, c], F32, kind="ExternalInput")
    bq = nc.dram_tensor("bq", [2 * cqk, 1], F32, kind="ExternalInput")
    bk = nc.dram_tensor("bk", [2 * cqk, 1], F32, kind="ExternalInput")
    ident = nc.dram_tensor("ident", [P, P], F32, kind="ExternalInput")
    out = nc.dram_tensor("out", [c, hwc], F32, kind="ExternalOutput")

    rgb_t = rgb.ap().rearrange("(k p) n -> p k n", p=P)
    chm_t = chm.ap().rearrange("(k p) n -> p k n", p=P)
    wq_t = wqT.ap().rearrange("(k p) m -> p k m", p=P)
    wk_t = wkT.ap().rearrange("(k p) m -> p k m", p=P)
    wv_t = wvT.ap().rearrange("(k p) m -> p k m", p=P)
    out_t = out.ap().rearrange("(k p) n -> p k n", p=P)

    with tile.TileContext(nc) as tc, \
         nc.allow_low_precision(reason="attention weights in fp8"):
        with tc.tile_pool(name="persist", bufs=1) as pers, \
             tc.tile_pool(name="ctpool", bufs=2) as ctpool:
            # --- persistent tiles ---
            wv_b = pers.tile([P, ck, c], BF16)
            bq_sb = pers.tile([2 * cqk, 1], F32)
            bk_sb = pers.tile([2 * cqk, 1], F32)
            qt_sb = pers.tile([2 * cqk, hwc], BF16)
            kf_sb = pers.tile([2 * cqk, xy], BF16)    # y-major columns
            wq_b = pers.tile([P, ck, 2 * cqk], BF16)
            wk_b = pers.tile([P, ck, 2 * cqk], BF16)
            id_b = pers.tile([P, P], BF16)
            rgb_b = pers.tile([P, ck, hwc], BF16)
            chmT8c = pers.tile([P, ck, jt, 2, P], FP8)  # chm^T, canonical

            # --- phase 1 ---
            with tc.tile_pool(name="qload", bufs=2) as qload, \
                 tc.tile_pool(name="kload", bufs=2) as kload, \
                 tc.tile_pool(name="wload", bufs=1) as wload, \
                 tc.tile_pool(name="c8pool", bufs=2) as c8pool:
                wq_sb = wload.tile([P, ck, 2 * cqk], F32, name="wq_sb")
                wk_sb = wload.tile([P, ck, 2 * cqk], F32, name="wk_sb")
                id_sb = wload.tile([P, P], F32, name="id_sb")
                nc.scalar.dma_start(wq_sb[:], wq_t)
                nc.scalar.dma_start(wk_sb[:], wk_t)
                nc.scalar.dma_start(bq_sb[:], bq.ap())
                nc.scalar.dma_start(bk_sb[:], bk.ap())
                rfh = [qload.tile([P, 2, hwc], F32, tag="rfh",
                                  name=f"rfh{h}") for h in range(2)]
                cfh = [kload.tile([P, 2, xy], F32, tag="cfh",
                                  name=f"cfh{h}") for h in range(2)]
                cb = {}
                ct8s = {}
                cf = [cfh[k // 2][:, k % 2] for k in range(ck)]

                def deint(k):
                    # de-interleave fp8 pairs to the canonical [K, s, M]
                    # weights layout (walrus rejects strided DR weights)
                    nc.gpsimd.tensor_copy(
                        chmT8c[:, k],
                        ct8s[k][:].bitcast(FP8)
                        .rearrange("p m (cc ss) -> p m ss cc", ss=2))

                wv_f = wload.tile([P, ck, c], F32)
                nc.scalar.dma_start(wv_f[:], wv_t)
                nc.scalar.dma_start(id_sb[:], ident.ap())
                nc.scalar.dma_start(cfh[0][:], chm_t[:, 0:2])
                nc.scalar.dma_start(rfh[0][:], rgb_t[:, 0:2])
                nc.scalar.dma_start(rfh[1][:], rgb_t[:, 2:4])
                nc.scalar.dma_start(cfh[1][:], chm_t[:, 2:4])
                nc.gpsimd.tensor_copy(wv_b[:], wv_f[:])
                nc.vector.tensor_copy(wq_b[:], wq_sb[:])
                nc.vector.tensor_copy(wk_b[:], wk_sb[:])
                nc.vector.tensor_copy(id_b[:], id_sb[:])
                for h in range(2):
                    # bf16 copy of rgb (DVE idle in phase 1)
                    nc.vector.tensor_copy(rgb_b[:, 2 * h:2 * h + 2],
                                          rfh[h][:])

                for k in range(ck):
                    # bf16 copies of chm chunk halves: K GEMM rhs + the
                    # chm^T transpose source (SP ring, as soon as ready)
                    cfv = cf[k][:].rearrange("p (xx yy) -> p yy xx", yy=y)
                    for h in range(2):
                        cb[k, h] = kload.tile([P, 32, nx], BF16, tag="cb",
                                              name=f"cb{k}_{h}")
                        nc.vector.tensor_copy(
                            cb[k, h][:], cfv[:, 32 * h:32 * (h + 1)])
                    # fp8 permuted copy (Pool), u16-pair transpose, then
                    # de-interleave to the canonical [K, s, M] weights
                    # layout on ACT (walrus rejects strided DR weights)
                    chm8k = c8pool.tile([P, xy], FP8, tag="chm8",
                                        name=f"chm8_{k}")
                    c8v = chm8k[:].rearrange("p (yy xx) -> p xx yy", xx=nx)
                    nc.gpsimd.tensor_copy(
                        c8v, cf[k][:].rearrange("p (xx yy) -> p xx yy",
                                                yy=y))
                    ct8s[k] = ctpool.tile([P, jt, P], U16, tag="ct8",
                                          name=f"ct8_{k}")
                    nc.sync.dma_start(ct8s[k][:], chm8k[:].bitcast(U16),
                                      transpose=True)
                    if k < 2:
                        deint(k)

                with tc.tile_pool(name="psQ", bufs=2, space="PSUM") as psQ:
                    # Q j-outer: one bank per 512-col block, 2 banks rotate
                    for j in range(nb):
                        q_ps = psQ.tile([2 * cqk, 512], F32, tag="qps",
                                        name="q_ps")
                        for k in range(ck):
                            nc.tensor.matmul(
                                q_ps[:], wq_b[:, k],
                                rgb_b[:, k, 512 * j:512 * (j + 1)],
                                start=(k == 0), stop=(k == ck - 1))
                        nc.scalar.activation(
                            qt_sb[:, 512 * j:512 * (j + 1)], q_ps[:],
                            IDENT, bias=bq_sb[:])

                with tc.tile_pool(name="psK", bufs=1, space="PSUM") as psK:
                    # K full-width k-outer in bf16 (8 banks), trailing the
                    # chm chunk loads + bf16 converts.
                    k_ps = [psK.tile([2 * cqk, 2048], F32, name=f"kps{i}")
                            for i in range(2)]
                    for k in range(ck):
                        for j in range(8):
                            nc.tensor.matmul(
                                k_ps[j // 4][:, 512 * (j % 4):
                                             512 * (j % 4 + 1)],
                                wk_b[:, k],
                                cb[k, j // 4][:]
                                .rearrange("p a b -> p (a b)")
                                [:, 512 * (j % 4):512 * (j % 4 + 1)],
                                start=(k == 0), stop=(k == ck - 1))
                            if k == ck - 1 and j % 2 == 1:
                                # drain each 1024-col group as soon as its
                                # accumulation completes so S(blk0) starts
                                # on the first columns early
                                jj = j // 2
                                nc.scalar.activation(
                                    kf_sb[:, 1024 * jj:1024 * (jj + 1)],
                                    k_ps[jj // 2][:, 1024 * (jj % 2):
                                                  1024 * (jj % 2 + 1)],
                                    IDENT, bias=bk_sb[:])

            # --- phase 2: softmax(b) / attend(b-2) pipeline ---
            with tc.tile_pool(name="pmain", bufs=4) as pmain, \
                 tc.tile_pool(name="pbpool", bufs=3) as pbpool, \
                 tc.tile_pool(name="p8pool", bufs=3) as p8pool, \
                 tc.tile_pool(name="ptpool", bufs=3) as ptpool, \
                 tc.tile_pool(name="zpool", bufs=1) as zpool, \
                 tc.tile_pool(name="opool", bufs=2) as opool, \
                 tc.tile_pool(name="m1pool", bufs=2) as m1pool, \
                 tc.tile_pool(name="psS", bufs=3, space="PSUM") as psS, \
                 tc.tile_pool(name="psA", bufs=1, space="PSUM") as psA:

                ptb = {}
                m1t = {}

                def m1_quarter(blk, ch, quarter):
                    """4 DoubleRow matmuls: one quarter of M1[ch, blk]."""
                    if blk not in m1t:
                        m1t[blk] = m1pool.tile([P, ck, 512], BF16, tag="m1",
                                               name=f"m1_{blk}")
                    if ch % 2 == 0 and quarter == 0:
                        m1t[(blk, "ps", ch // 2)] = psA.tile(
                            [P, 1024], F32, tag="aps", name="m1ps")
                    m_ps = m1t[(blk, "ps", ch // 2)]
                    half = m_ps[:, 512 * (ch % 2):512 * (ch % 2 + 1)]
                    for m in range(4 * quarter, 4 * quarter + 4):
                        rhs = (ptb[blk][:, m]
                               .rearrange("p hh nn -> p (hh nn)")
                               .bitcast(FP8)
                               .rearrange("p (nn ss) -> p ss nn", ss=2))
                        nc.tensor.matmul(
                            half, chmT8c[:, ch, m], rhs, perf_mode=DR,
                            start=(m == 0), stop=(m == jt - 1))
                    if ch % 2 == 1 and quarter == 3:
                        m_ps = m1t.pop((blk, "ps", ch // 2))
                        nc.vector.tensor_copy(
                            m1t[blk][:, ch - 1:ch + 1], m_ps[:]
                            .rearrange("p (a b) -> p a b", a=2))

                def softmax_htile(blk, ht, interleave, pool_convert):
                    """S matmuls + exp + normalize + fp8 transpose for one
                    128-row hw tile; S-chunk matmuls alternate with
                    `interleave` chunks (callables emitting PE work)."""
                    htile = blk * 4 + ht
                    p_sb = pmain.tile([P, xy], BF16, tag="p")
                    for s in range(xy // 1024):
                        s_ps = psS.tile([P, 1024], F32, tag="sps")
                        nc.tensor.matmul(
                            s_ps[:, 0:512],
                            qt_sb[0:cqk, P * htile:P * (htile + 1)],
                            kf_sb[0:cqk, 1024 * s:1024 * s + 512],
                            start=True, stop=True, tile_position=(0, 0))
                        nc.tensor.matmul(
                            s_ps[:, 512:1024],
                            qt_sb[cqk:2 * cqk, P * htile:P * (htile + 1)],
                            kf_sb[cqk:2 * cqk, 1024 * s + 512:1024 * (s + 1)],
                            start=True, stop=True, tile_position=(cqk, 0))
                        nc.scalar.activation(
                            p_sb[:, 1024 * s:1024 * (s + 1)], s_ps[:], EXP)
                        if s < len(interleave):
                            interleave[s]()
                    # Z[hw, x] = sum_y P_u[hw, y, x]: pairwise tree over the
                    # y (outer) dim keeps stride-1 innermost (DVE 2x).
                    v3 = p_sb[:].rearrange("p (yy xx) -> p yy xx", xx=nx)
                    tcur = v3
                    w = y
                    while w > 1:
                        w //= 2
                        tnext = zpool.tile([P, w, nx], BF16, tag=f"z{w}")
                        nc.vector.tensor_tensor(
                            tnext[:], tcur[:, 0:w], tcur[:, w:2 * w], ADD)
                        tcur = tnext
                    rz = zpool.tile([P, 1, nx], BF16, tag="rz")
                    nc.vector.reciprocal(rz[:], tcur[:])
                    p8 = p8pool.tile([P, y, nx], FP8, tag="p8")
                    if pool_convert:
                        # bf16 normalize at DVE 2x; Pool quantizes to fp8
                        pb = pbpool.tile([P, y, nx], BF16, tag="pb")
                        nc.vector.tensor_tensor(
                            pb[:], v3, rz[:].to_broadcast([P, y, nx]), MULT)
                        nc.gpsimd.tensor_copy(p8[:], pb[:])
                    else:
                        # normalize + quantize to fp8 in one DVE pass
                        nc.vector.tensor_tensor(
                            p8[:], v3, rz[:].to_broadcast([P, y, nx]), MULT)
                    if blk not in ptb:
                        # [p, m, ht, hw]: hw contiguous across ht per m so
                        # the M1 rhs merges to one 512-wide moving AP.
                        ptb[blk] = ptpool.tile([P, jt, 4, P], U16,
                                               tag="ptb", name=f"ptb{blk}")
                    nc.sync.dma_start(ptb[blk][:, :, ht], p8[:].bitcast(U16),
                                      transpose=True)

                def attend_fini(blk):
                    """att2 + rgb add (via identity matmul) + store."""
                    m1_sb = m1t.pop(blk)
                    o_sb = opool.tile([P, ck, 512], F32, tag="o")
                    for cp in range(2):
                        a_ps = psA.tile([P, 1024], F32, tag="aps",
                                        name="a_ps")
                        for h in range(2):
                            ct = 2 * cp + h
                            half = a_ps[:, 512 * h:512 * (h + 1)]
                            for chh in range(ck):
                                nc.tensor.matmul(
                                    half, wv_b[:, chh, P * ct:P * (ct + 1)],
                                    m1_sb[:, chh],
                                    start=(chh == 0), stop=False)
                            nc.tensor.matmul(
                                half, id_b[:],
                                rgb_b[:, ct, 512 * blk:512 * (blk + 1)],
                                start=False, stop=True)
                        nc.vector.tensor_copy(
                            o_sb[:, 2 * cp:2 * cp + 2],
                            a_ps[:].rearrange("p (a b) -> p a b", a=2))
                        # store each half as soon as its drain lands
                        nc.sync.dma_start(
                            out_t[:, 2 * cp:2 * cp + 2,
                                  512 * blk:512 * (blk + 1)],
                            o_sb[:, 2 * cp:2 * cp + 2])
                    del ptb[blk]

                def slot(ab, ch):
                    return [(lambda c=ch, q=q: m1_quarter(ab, c, q))
                            for q in range(4)]

                # depth-1.5 pipeline: softmax(b) hosts the second half of
                # attend(b-2), its fini, then the first half of attend(b-1)
                for blk in range(nb):
                    for ht in range(4):
                        ab, ch = blk - 2 + ht // 2, (ht % 2) + 2 * (ht // 2 == 0)
                        ivl = slot(ab, ch) if 0 <= ab < nb else []
                        softmax_htile(blk, ht, ivl, pool_convert=ht < 3)
                        if blk == 1 and ht < 2:
                            deint(ht + 2)
                        if blk >= 2 and ht == 1:
                            attend_fini(blk - 2)
                # tail: finish attend(nb-2) and attend(nb-1)
                for ch in range(2, ck):
                    for q in range(4):
                        m1_quarter(nb - 2, ch, q)
                attend_fini(nb - 2)
                for ch in range(ck):
                    for q in range(4):
                        m1_quarter(nb - 1, ch, q)
                attend_fini(nb - 1)

    nc.compile()
    return nc


_NC_CACHE = {}


def _get_nc():
    if "nc" not in _NC_CACHE:
        _NC_CACHE["nc"] = build_program()
    return _NC_CACHE["nc"]


def make_in_maps(rgb_features, chm_features, Wq, bq, Wk, bk, Wv, bv, gamma):
    rgb_features = np.asarray(rgb_features, dtype=np.float32)
    chm_features = np.asarray(chm_features, dtype=np.float32)
    Wq = np.asarray(Wq, dtype=np.float32)
    Wk = np.asarray(Wk, dtype=np.float32)
    Wv = np.asarray(Wv, dtype=np.float32)
    bq = np.asarray(bq, dtype=np.float32)
    bk = np.asarray(bk, dtype=np.float32)
    bv = np.asarray(bv, dtype=np.float32)
    g = float(np.asarray(gamma).reshape(-1)[0])

    wqT = np.ascontiguousarray(np.concatenate([Wq.T, Wq.T], axis=1))
    wkT = np.ascontiguousarray(np.concatenate([Wk.T, Wk.T], axis=1))
    wvT = np.ascontiguousarray((g * Wv).T)
    # softmax rows sum to 1 per (hw, x); summing over the 64 x's makes the
    # bias term contribute exactly 64*gamma*bv[c] to every output pixel.
    rgb_adj = rgb_features + (64.0 * g * bv)[None, :, None, None]
    bq2 = np.ascontiguousarray(np.concatenate([bq, bq]).reshape(2 * CQK, 1))
    bk2 = np.ascontiguousarray(np.concatenate([bk, bk]).reshape(2 * CQK, 1))
    ident = np.eye(P, dtype=np.float32)

    in_maps = []
    for core in range(N_CORES):
        b, half = divmod(core, 2)
        rgb_c = np.ascontiguousarray(
            rgb_adj[b].reshape(C, HW)[:, half * HWC:(half + 1) * HWC])
        chm_c = np.ascontiguousarray(chm_features[b].reshape(C, HW))
        in_maps.append({
            "rgb": rgb_c, "chm": chm_c,
            "wqT": wqT, "wkT": wkT, "wvT": wvT,
            "bq": bq2, "bk": bk2, "ident": ident,
        })
    return in_maps


def assemble(results):
    fused = np.empty((B, C, H, W), dtype=np.float32)
    fused2 = fused.reshape(B, C, HW)
    for core in range(N_CORES):
        b, half = divmod(core, 2)
        fused2[b, :, half * HWC:(half + 1) * HWC] = results[core]["out"]
    return fused


def kernel(rgb_features, chm_features, Wq, bq, Wk, bk, Wv, bv, gamma):
    nc = _get_nc()
    in_maps = make_in_maps(rgb_features, chm_features, Wq, bq, Wk, bk, Wv, bv,
                           gamma)
    res = run_bass_kernel_spmd(nc, in_maps, core_ids=list(range(N_CORES)))
    return assemble(res.results)
